# revision 21
# baseline (speedup 1.0000x reference)
"""Trainium2 Bass kernel for nn_NeuralProgramSynthesis (moe_routing).

Strategy: data-parallel over batch (16 images per core, 8 cores, no
collectives). Per core:
  - gating: mean-pool features with a sliding-window ones matmul accumulated
    into one [nb, 512] PSUM tile, 2-layer MLP (fp32r matmuls, PE transposes),
    softmax over 16 ops per step, then build per-(step,batch) block-diagonal
    probability matrices via a diagonal-strided DMA roundtrip through DRAM.
  - 8 program steps, fully unrolled: the 10-channel 64x64 grid lives in SBUF
    as a 66x66 zero-padded fp32r image per batch; a [90, 4224] im2col replica
    (9 shifted copies) is rebuilt per batch per step with 9 SBUF->SBUF DMAs;
    conv1 = K=90 fp32r matmuls (4 expert groups of 128 out channels),
    relu (ACT/DVE), conv2 = block-diagonal K=128 -> M=64 matmuls (2 groups
    packed per PSUM tile at partition offsets 0/64), tanh (ACT), expert
    combine = 2 accumulating K=128 matmuls against the block-probability
    weights. Combine output is evacuated PSUM->SBUF by ACT/DVE copies, then
    DMA'd to the trace/final outputs and back into the padded SBUF grid.
"""

import contextlib

import numpy as np

import concourse.bass as bass
import concourse.tile as tile
from concourse import bacc, mybir
from concourse.bass_utils import run_bass_kernel_spmd

F32 = mybir.dt.float32
F32R = mybir.dt.float32r
AF = mybir.ActivationFunctionType
AX = mybir.AxisListType

N_CORES = 8
H = W = 64
PW = 66
NPIX = PW * PW          # 4356
I2W = 4224              # im2col tile width (>= 63*66+63+1 = 4222)
C = 10
E = 16
DM = 512
SP = 900
FULL_NB = 16
FULL_NSTEPS = 8


def _ap(base, extra_off, dims):
    return bass.AP(base.tensor, base.offset + extra_off, dims)


def build(nb=FULL_NB, nsteps=FULL_NSTEPS):
    assert nb <= 16
    nc = bacc.Bacc("TRN2", target_bir_lowering=False, debug=False)

    feat = nc.dram_tensor("feat", [nb, SP, DM], F32R, kind="ExternalInput").ap()
    grid0 = nc.dram_tensor("grid0", [nb, C, H, W], F32R, kind="ExternalInput").ap()
    w1p = nc.dram_tensor("w1p", [4, 90, 128], F32R, kind="ExternalInput").ap()
    w2bd = nc.dram_tensor("w2bd", [4, 128, 128], F32R, kind="ExternalInput").ap()
    wg1 = nc.dram_tensor("wg1", [4, 8, 128, 128], F32R, kind="ExternalInput").ap()
    wg2 = nc.dram_tensor("wg2", [8, 128, 128], F32R, kind="ExternalInput").ap()
    ident = nc.dram_tensor("ident", [128, 128], F32, kind="ExternalInput").ap()
    onesc = nc.dram_tensor("onesc", [128, 2 * nb - 1], F32R, kind="ExternalInput").ap()
    bg1c = nc.dram_tensor("bg1c", [128, 8], F32, kind="ExternalInput").ap()
    bg2c = nc.dram_tensor("bg2c", [128, 1], F32, kind="ExternalInput").ap()
    b1c = nc.dram_tensor("b1c", [128, 4], F32, kind="ExternalInput").ap()
    b2c = nc.dram_tensor("b2c", [128, 2], F32, kind="ExternalInput").ap()

    rsel = nc.dram_tensor("rsel", [nsteps * 2, 128, 128], F32R, kind="ExternalInput").ap()
    maskc = nc.dram_tensor("maskc", [128, 10], F32, kind="ExternalInput").ap()

    probs_o = nc.dram_tensor("probs_o", [nb, nsteps, E], F32, kind="ExternalOutput").ap()
    trace_o = nc.dram_tensor("trace_o", [nb, nsteps, C, H, W], F32R, kind="ExternalOutput").ap()
    final_o = nc.dram_tensor("final_o", [nb, C, H, W], F32R, kind="ExternalOutput").ap()

    ngrp = (nb + 11) // 12
    grp_nb = [min(12, nb - 12 * k) for k in range(ngrp)]

    with tile.TileContext(nc) as tc:
        with contextlib.ExitStack() as ctx:
            # ---------------- persistent SBUF pools ----------------
            wpool = ctx.enter_context(tc.tile_pool(name="wpool", bufs=1))
            gpad_pool = ctx.enter_context(tc.tile_pool(name="gpad", bufs=1))
            i2c_pool = ctx.enter_context(tc.tile_pool(name="i2c", bufs=3))
            hsb_pool = ctx.enter_context(tc.tile_pool(name="hsb", bufs=3))
            tsb_pool = ctx.enter_context(tc.tile_pool(name="tsb", bufs=3))
            fsb_pool = ctx.enter_context(tc.tile_pool(name="fsb", bufs=3))

            # ---------------- weights / consts ----------------
            w1sb = wpool.tile([90, 4 * 128], F32R, tag="w1sb")
            nc.sync.dma_start(w1sb[:], _ap(w1p, 0, [[128, 90], [90 * 128, 4], [1, 128]]))
            w2sb = wpool.tile([128, 4 * 128], F32R, tag="w2sb")
            nc.sync.dma_start(w2sb[:], _ap(w2bd, 0, [[128, 128], [128 * 128, 4], [1, 128]]))
            wg1sb = wpool.tile([128, 32 * 128], F32R, tag="wg1sb")
            nc.sync.dma_start(
                wg1sb[:], _ap(wg1, 0, [[128, 128], [8 * 128 * 128, 4], [128 * 128, 8], [1, 128]])
            )
            wg2sb = wpool.tile([128, 8 * 128], F32R, tag="wg2sb")
            nc.sync.dma_start(
                wg2sb[:], _ap(wg2, 0, [[128, 128], [128 * 128, 8], [1, 128]])
            )
            idsb = wpool.tile([128, 128], F32, tag="idsb")
            nc.sync.dma_start(idsb[:], ident[:])
            onesb = wpool.tile([128, 2 * nb - 1], F32R, tag="onesb")
            nc.sync.dma_start(onesb[:], onesc[:])
            bg1sb = wpool.tile([128, 8], F32, tag="bg1sb")
            nc.sync.dma_start(bg1sb[:], bg1c[:])
            bg2sb = wpool.tile([128, 1], F32, tag="bg2sb")
            nc.sync.dma_start(bg2sb[:], bg2c[:])
            b1sb = wpool.tile([128, 4], F32, tag="b1sb")
            nc.sync.dma_start(b1sb[:], b1c[:])
            b2sb = wpool.tile([128, 2], F32, tag="b2sb")
            nc.sync.dma_start(b2sb[:], b2c[:])
            rsb = wpool.tile([128, nsteps * 2 * 128], F32R, tag="rsb")
            nc.sync.dma_start(
                rsb[:], _ap(rsel, 0, [[128, 128], [128 * 128, nsteps * 2], [1, 128]])
            )
            masksb = wpool.tile([128, 10], F32, tag="masksb")
            nc.sync.dma_start(masksb[:], maskc[:])
            bp_all = wpool.tile([128, nsteps * nb * 2 * 10], F32R, tag="bp_all")

            # ---------------- padded grids + initial fill ----------------
            gp = []
            for k in range(ngrp):
                t = gpad_pool.tile([grp_nb[k] * 10, NPIX], F32R, tag=f"gp{k}")
                nc.gpsimd.memset(t[:].bitcast(F32), 0.0)
                gp.append(t)

            def gpad_slice(b, free_off, free_dims):
                k, bb = divmod(b, 12)
                base = gp[k][:]
                return bass.AP(
                    base.tensor,
                    base.offset + (bb * 10) * NPIX + free_off,
                    [[NPIX, 10]] + free_dims,
                )

            for b in range(nb):
                nc.sync.dma_start(
                    gpad_slice(b, PW + 1, [[PW, H], [1, W]]),
                    grid0[b],
                )

            # ---------------- im2col builds ----------------
            i2c_of = {}

            def build_i2c(s, b):
                t = i2c_pool.tile([90, I2W], F32R, tag="i2c")
                i2c_of[(s, b)] = t
                for tap in range(9):
                    dy, dx = divmod(tap, 3)
                    off = dy * PW + dx
                    nc.sync.dma_start(
                        t[tap * 10:(tap + 1) * 10, 0:4222],
                        gpad_slice(b, off, [[1, 4222]]),
                    )
                return t

            for b in range(nb):
                build_i2c(0, b)

            # ---------------- gating ----------------
            with tc.tile_pool(name="gat_sb", bufs=1) as gsb_pool, \
                 tc.tile_pool(name="gat_fd", bufs=4) as gfd_pool, \
                 tc.tile_pool(name="gat_ps", bufs=2, space="PSUM") as gps_pool, \
                 tc.tile_pool(name="gat_ps1", bufs=1, space="PSUM") as gps1_pool:
                gsb = gsb_pool.tile([nb, DM], F32, tag="gsb")
                gtsb = gsb_pool.tile([128, 4 * nb], F32R, tag="gtsb")
                h1sb = gsb_pool.tile([128, 8 * nb], F32R, tag="h1sb")
                ltsb = gsb_pool.tile([128, nb], F32, tag="ltsb")
                lgsb = gsb_pool.tile([nb, 128], F32, tag="lgsb")
                essb = gsb_pool.tile([nb, 128], F32, tag="essb")
                pssb = gsb_pool.tile([nb, 128], F32, tag="pssb")
                pstsb = gsb_pool.tile([128, nb], F32R, tag="pstsb")
                pvsb = gsb_pool.tile([128, nsteps * nb * 2], F32, tag="pvsb")
                mxsb = gsb_pool.tile([nb, 2 * nsteps], F32, tag="mxsb")

                # mean pool: all batches accumulate into one [nb, DM] psum
                # via a sliding-window ones lhsT (col b is all-ones for
                # batch b's row-chunks, zero elsewhere).
                gsum = gps1_pool.tile([nb, DM], F32, tag="gps1")
                nmm = 0
                for b in range(nb):
                    for kk in range(8):
                        rows = 128 if kk < 7 else SP - 7 * 128
                        ft = gfd_pool.tile([128, DM], F32R, tag="ft")
                        nc.sync.dma_start(
                            ft[0:rows, :],
                            _ap(feat, (b * SP + kk * 128) * DM, [[DM, rows], [1, DM]]),
                        )
                        nc.tensor.matmul(
                            gsum[:],
                            onesb[0:rows, nb - 1 - b:2 * nb - 1 - b],
                            ft[0:rows, :],
                            start=(nmm == 0), stop=(nmm == 8 * nb - 1),
                        )
                        nmm += 1
                nc.scalar.activation(gsb[:], gsum[:], AF.Identity)

                # transpose g -> [128, nb] x4 chunks
                for kk in range(4):
                    gt_ps = gps_pool.tile([128, nb], F32, tag="gt_ps")
                    nc.tensor.transpose(
                        gt_ps[:], gsb[:, kk * 128:(kk + 1) * 128], idsb[0:nb, 0:nb]
                    )
                    nc.scalar.activation(
                        gtsb[:, kk * nb:(kk + 1) * nb], gt_ps[:], AF.Identity
                    )

                # mm1: h1 pre-activation chunks [128, nb]
                h1pre = gsb_pool.tile([128, 8 * nb], F32, tag="h1pre")
                for hc in range(8):
                    h1ps = gps_pool.tile([128, nb], F32, tag="h1ps")
                    for kk in range(4):
                        nc.tensor.matmul(
                            h1ps[:],
                            wg1sb[:, (kk * 8 + hc) * 128:(kk * 8 + hc + 1) * 128],
                            gtsb[:, kk * nb:(kk + 1) * nb],
                            start=(kk == 0), stop=(kk == 3),
                        )
                    nc.scalar.activation(
                        h1pre[:, hc * nb:(hc + 1) * nb], h1ps[:], AF.Identity,
                        bias=bg1sb[:, hc:hc + 1],
                    )
                # tanh-form GELU (exact to <1e-6 for |x| < 0.5, which holds
                # here since pooled-feature activations are tiny):
                # gelu(x) = 0.5 x (1 + tanh(0.7978845608 (x + 0.044715 x^3)))
                gtmp1 = gsb_pool.tile([128, 8 * nb], F32, tag="gtmp1")
                gtmp2 = gsb_pool.tile([128, 8 * nb], F32, tag="gtmp2")
                gxh = gsb_pool.tile([128, 8 * nb], F32, tag="gxh")
                nc.vector.tensor_mul(gtmp1[:], h1pre[:], h1pre[:])
                nc.vector.tensor_mul(gtmp2[:], gtmp1[:], h1pre[:])
                nc.vector.scalar_tensor_tensor(
                    gtmp1[:], gtmp2[:], 0.044715, h1pre[:],
                    op0=mybir.AluOpType.mult, op1=mybir.AluOpType.add,
                )
                nc.vector.tensor_scalar_mul(gxh[:], h1pre[:], 0.5)
                nc.scalar.activation(
                    gtmp2[:], gtmp1[:], AF.Tanh, scale=0.7978845608028654
                )
                nc.vector.scalar_tensor_tensor(
                    h1sb[:], gtmp2[:], 1.0, gxh[:],
                    op0=mybir.AluOpType.add, op1=mybir.AluOpType.mult,
                )

                # mm2: logits_T [128, nb]
                ltps = gps1_pool.tile([128, nb], F32, tag="gps1")
                for hc in range(8):
                    nc.tensor.matmul(
                        ltps[:],
                        wg2sb[:, hc * 128:(hc + 1) * 128],
                        h1sb[:, hc * nb:(hc + 1) * nb],
                        start=(hc == 0), stop=(hc == 7),
                    )
                nc.scalar.activation(ltsb[:], ltps[:], AF.Identity, bias=bg2sb[:])

                # transpose logits -> [nb, 128]
                lgps = gps1_pool.tile([nb, 128], F32, tag="gps1")
                nc.tensor.transpose(lgps[:], ltsb[:], idsb[:])
                nc.scalar.activation(lgsb[:], lgps[:], AF.Identity)

                # softmax over each step's 16 ops (free-dim groups)
                for s in range(nsteps):
                    sl = lgsb[:, s * 16:(s + 1) * 16]
                    mx = mxsb[:, 2 * s:2 * s + 1]
                    sm = mxsb[:, 2 * s + 1:2 * s + 2]
                    nc.vector.reduce_max(out=mx, in_=sl, axis=AX.X, negate=True)
                    nc.scalar.activation(
                        essb[:, s * 16:(s + 1) * 16], sl, AF.Exp,
                        bias=mx, accum_out=sm,
                    )
                    nc.vector.reciprocal(out=sm, in_=sm)
                    nc.vector.tensor_scalar_mul(
                        pssb[:, s * 16:(s + 1) * 16],
                        essb[:, s * 16:(s + 1) * 16],
                        sm,
                    )
                nc.sync.dma_start(probs_o[:], pssb[:, 0:nsteps * 16])

                # block-probability build, all on-device:
                #  1. transpose probs -> [128 L, nb]
                #  2. per (step, half): one-hot selector matmul replicates
                #     p[b, s, e] onto partition rows (e%8)*16+cc
                #  3. one broadcast tensor_mul against the column mask forms
                #     all [128, 10] block matrices at once.
                nL = nsteps * 16
                pstps = gps1_pool.tile([128, nb], F32, tag="gps1")
                nc.tensor.transpose(
                    pstps[0:nL, :], pssb[:, 0:nL], idsb[0:nb, 0:nb]
                )
                nc.scalar.activation(pstsb[0:nL, :], pstps[0:nL, :], AF.Identity)

                pvcols = nsteps * nb * 2
                pvbase = pvsb[:]
                for si in range(nsteps * 2):
                    s, hf = divmod(si, 2)
                    pvps = gps_pool.tile([128, nb], F32, tag="pvps")
                    nc.tensor.matmul(
                        pvps[:],
                        rsb[0:nL, si * 128:(si + 1) * 128],
                        pstsb[0:nL, :],
                        start=True, stop=True,
                    )
                    nc.scalar.activation(
                        bass.AP(
                            pvbase.tensor,
                            pvbase.offset + s * nb * 2 + hf,
                            [[pvcols, 128], [2, nb]],
                        ),
                        pvps[:], AF.Identity,
                    )
                mbase = masksb[:]
                nc.vector.tensor_mul(
                    bp_all[:],
                    bass.AP(pvbase.tensor, pvbase.offset,
                            [[pvcols, 128], [1, pvcols], [0, 10]]),
                    bass.AP(mbase.tensor, mbase.offset,
                            [[10, 128], [0, pvcols], [1, 10]]),
                )

            # ---------------- steps ----------------
            # PSUM pools opened only after the gating pools closed:
            # hp 3x1 + tp 2x2 + fp 1x1 = 8 banks.
            hp_pool = ctx.enter_context(tc.tile_pool(name="hp", bufs=3, space="PSUM"))
            tp_pool = ctx.enter_context(tc.tile_pool(name="tp", bufs=2, space="PSUM"))
            fp_pool = ctx.enter_context(tc.tile_pool(name="fp", bufs=1, space="PSUM"))
            relu_rr = [0]

            def relu_op(dst, src, g):
                i = relu_rr[0]
                relu_rr[0] += 1
                if i % 4 == 3:
                    nc.scalar.activation(dst, src, AF.Relu, bias=b1sb[:, g:g + 1])
                else:
                    nc.vector.tensor_scalar(
                        dst, src, b1sb[:, g:g + 1], 0.0,
                        op0=mybir.AluOpType.add, op1=mybir.AluOpType.max,
                    )

            for s in range(nsteps):
                for b in range(nb):
                    i2c = i2c_of.pop((s, b))
                    ibase = i2c[:]
                    for blk in range(4):
                        y0 = blk * 16
                        hsb = []
                        tps = []

                        def conv2(g):
                            # M=128 with the odd group's weights zero-padded
                            # into columns 64:128; the two groups accumulate
                            # into one [128, 1024] PSUM tile (zeros add
                            # harmlessly), keeping dst partition 0 (ISA rule).
                            tp = tps[g // 2]
                            for c2 in range(2):
                                nc.tensor.matmul(
                                    tp[:, c2 * 512:(c2 + 1) * 512],
                                    w2sb[:, g * 128:(g + 1) * 128],
                                    hsb[g][:, c2 * 512:(c2 + 1) * 512],
                                    start=(g % 2 == 0), stop=(g % 2 == 1),
                                )

                        for g in range(4):
                            hs = hsb_pool.tile([128, 1024], F32R, tag="hsb")
                            hsb.append(hs)
                            if g % 2 == 0:
                                tpt = tp_pool.tile([128, 1024], F32, tag="tp")
                                tps.append(tpt)
                            for c2 in range(2):
                                hp = hp_pool.tile([128, 512], F32, tag="hp")
                                rhs = bass.AP(
                                    ibase.tensor,
                                    ibase.offset + (y0 + 8 * c2) * PW,
                                    [[I2W, 90], [PW, 8], [1, W]],
                                )
                                nc.tensor.matmul(
                                    hp[:], w1sb[:, g * 128:(g + 1) * 128], rhs,
                                    start=True, stop=True,
                                )
                                relu_op(hs[:, c2 * 512:(c2 + 1) * 512], hp[:], g)
                            if g >= 1:
                                conv2(g - 1)
                        conv2(3)

                        tsb = []
                        for hf in range(2):
                            ts = tsb_pool.tile([128, 1024], F32R, tag="tsb")
                            tsb.append(ts)
                            nc.scalar.activation(
                                ts[:], tps[hf][:], AF.Tanh, bias=b2sb[:, hf:hf + 1]
                            )

                        fs = fsb_pool.tile([C, 1024], F32R, tag="fsb")
                        for c2 in range(2):
                            fp = fp_pool.tile([C, 512], F32, tag="fp")
                            boff = ((s * nb + b) * 2) * 10
                            nc.tensor.matmul(
                                fp[:], bp_all[:, boff:boff + 10],
                                tsb[0][:, c2 * 512:(c2 + 1) * 512],
                                start=True, stop=False,
                            )
                            nc.tensor.matmul(
                                fp[:], bp_all[:, boff + 10:boff + 20],
                                tsb[1][:, c2 * 512:(c2 + 1) * 512],
                                start=False, stop=True,
                            )
                            if c2 == 0:
                                nc.vector.tensor_copy(
                                    fs[:, 0:512], fp[:]
                                )
                            else:
                                nc.scalar.activation(
                                    fs[:, 512:1024], fp[:], AF.Identity
                                )
                        # one DMA per destination per 16-row block
                        nc.sync.dma_start(
                            trace_o[b, s, :, y0:y0 + 16, :], fs[:]
                        )
                        nc.sync.dma_start(
                            gpad_slice(b, (y0 + 1) * PW + 1, [[PW, 16], [1, W]]),
                            fs[:],
                        )
                        if s == nsteps - 1:
                            nc.sync.dma_start(
                                final_o[b, :, y0:y0 + 16, :], fs[:]
                            )
                    if s < nsteps - 1:
                        build_i2c(s + 1, b)

    nc.compile()
    return nc


def _prep_weights(Wg1, bg1, Wg2, bg2, W1, b1, W2, b2, nb=FULL_NB,
                  nsteps=FULL_NSTEPS):
    f32 = np.float32
    a = W1.transpose(3, 4, 2, 0, 1).reshape(90, 16, 32)
    w1p = np.ascontiguousarray(
        a.reshape(90, 4, 4 * 32).transpose(1, 0, 2)
    ).astype(f32)

    w2bd = np.zeros((4, 128, 128), f32)
    w2s = W2[:, :, :, 0, 0]  # [e, cout10, cin32]
    for g in range(4):
        po = 64 * (g % 2)
        for q in range(4):
            w2bd[g, q * 32:(q + 1) * 32, po + q * 16:po + q * 16 + 10] = w2s[4 * g + q].T

    wg1 = np.ascontiguousarray(
        (Wg1 / SP).reshape(4, 128, 8, 128).transpose(0, 2, 1, 3)
    ).astype(f32)
    wg2 = np.ascontiguousarray(Wg2.reshape(8, 128, 128)).astype(f32)

    bg1c = np.ascontiguousarray(bg1.reshape(8, 128).T).astype(f32)
    bg2c = np.ascontiguousarray(bg2.reshape(128, 1)).astype(f32)
    b1c = np.ascontiguousarray(
        b1.reshape(4, 4, 32).transpose(1, 2, 0).reshape(128, 4)
    ).astype(f32)
    b2c = np.zeros((2, 2, 4, 16), f32)
    for hf in range(2):
        for gg in range(2):
            for q in range(4):
                b2c[hf, gg, q, :10] = b2[4 * (2 * hf + gg) + q]
    b2c = np.ascontiguousarray(b2c.reshape(2, 128).T)

    onesc = np.zeros((128, 2 * nb - 1), f32)
    onesc[:, nb - 1] = 1.0

    # rsel[(s,hf), L, r] = 1 iff L == s*16 + hf*8 + r//16
    rsel = np.zeros((nsteps * 2, 128, 128), f32)
    rr = np.arange(128)
    for s in range(nsteps):
        for hf in range(2):
            rsel[s * 2 + hf, s * 16 + hf * 8 + rr // 16, rr] = 1.0
    # maskc[r, cc'] = 1 iff r%16 == cc' (< 10)
    maskc = np.zeros((128, 10), f32)
    for r in range(128):
        if r % 16 < 10:
            maskc[r, r % 16] = 1.0

    return dict(
        w1p=w1p, w2bd=w2bd, wg1=wg1, wg2=wg2,
        ident=np.eye(128, dtype=f32), onesc=onesc, rsel=rsel, maskc=maskc,
        bg1c=bg1c, bg2c=bg2c, b1c=b1c, b2c=b2c,
    )


_NC_CACHE = {}


def _get_nc(nb=FULL_NB, nsteps=FULL_NSTEPS):
    key = (nb, nsteps)
    if key not in _NC_CACHE:
        _NC_CACHE[key] = build(nb, nsteps)
    return _NC_CACHE[key]


def make_in_maps(features, input_grid, Wg1, bg1, Wg2, bg2, W1, b1, W2, b2,
                 nb=FULL_NB, nsteps=FULL_NSTEPS, n_cores=N_CORES):
    wd = _prep_weights(Wg1, bg1, Wg2, bg2, W1, b1, W2, b2, nb=nb, nsteps=nsteps)
    f32 = np.float32
    feat_all = np.ascontiguousarray(
        features.reshape(features.shape[0], SP, DM)
    ).astype(f32)
    grid_all = np.ascontiguousarray(input_grid).astype(f32)
    in_maps = []
    for i in range(n_cores):
        b0 = i * nb
        m = dict(wd)
        m["feat"] = feat_all[b0:b0 + nb]
        m["grid0"] = grid_all[b0:b0 + nb]
        in_maps.append(m)
    return in_maps


def kernel(features, input_grid, Wg1, bg1, Wg2, bg2, W1, b1, W2, b2):
    nc = _get_nc()
    in_maps = make_in_maps(
        features, input_grid, Wg1, bg1, Wg2, bg2, W1, b1, W2, b2
    )
    br = run_bass_kernel_spmd(nc, in_maps, list(range(N_CORES)))
    final = np.concatenate([br.results[i]["final_o"] for i in range(N_CORES)], 0)
    probs = np.concatenate([br.results[i]["probs_o"] for i in range(N_CORES)], 0)
    trace = np.concatenate([br.results[i]["trace_o"] for i in range(N_CORES)], 0)
    return final, probs, trace
